# revision 4
# baseline (speedup 1.0000x reference)
"""Trainium2 Bass kernel for the segment-reduce KD loss.

Math (see derivation in comments below):
  per-class/per-batch/per-channel masked sums s_sum[k,b,c], t_sum[k,b,c]
  (c in 1..4; channel 0 is dropped by the loss so it is never loaded),
  nvox[k] = voxel count of class k over the whole batch, then
    a_s = s_sum / (nvox+eps) / T,  a_t = t_sum / (nvox+eps) / T
    kl_st + kl_ts  =  A_t/Z_t - A_s/Z_s          (the log-sum-exp terms cancel)
  where, per (k, b):  Z_x = sum_c exp(a_x[c]),  A_x = sum_c exp(a_x[c]) * (a_t[c]-a_s[c])
  final = sum_{k,b} (A_t/Z_t - A_s/Z_s) / (2 * B * K)

Distribution: data-parallel over the D axis (96 -> 12 per core, 8 cores).
Each core computes partial sums via PE matmuls (onehot.T @ [s|t|ones]) with
the contraction over 128 spatial slabs on the partition axis, then the tiny
[5 x 20] partials are AllGathered and every core finishes the scalar loss.
"""

import numpy as np

import concourse.bacc as bacc
import concourse.mybir as mybir
import concourse.tile as tile
from concourse.bass_utils import run_bass_kernel_spmd

F32 = mybir.dt.float32
I32 = mybir.dt.int32
AX = mybir.AxisListType
OP = mybir.AluOpType

N_CORES = 8
B, NCLS, D, H, W = 2, 5, 96, 160, 160
K = NCLS          # segment classes
C = NCLS - 1      # channels used (1..4)
CD = 2 * C        # data columns per voxel: 4 src + 4 tgt
NCOL = CD + 1     # + ones column (voxel counts)
P = 128
D_CORE = D // N_CORES
S_CORE = D_CORE * H * W          # 307200 spatial elements per core per batch
TEMP = 2.0
EPS = 1e-6


def _build(s_core=S_CORE, fb=1200, n_cg=4):
    """Build + compile the 8-core SPMD Bass program.

    s_core: per-core spatial size (must be divisible by 128*fb)
    fb:     f-block size (columns per SBUF tile)
    n_cg:   PE column groups used for concurrent matmuls (1, 2 or 4)
    """
    ftot = s_core // P
    nblk = ftot // fb
    assert ftot % fb == 0 and fb % n_cg == 0

    nc = bacc.Bacc("TRN2", target_bir_lowering=False, debug=False,
                   num_devices=N_CORES)
    src = nc.dram_tensor("src", [B, C, s_core], F32, kind="ExternalInput")
    tgt = nc.dram_tensor("tgt", [B, C, s_core], F32, kind="ExternalInput")
    gti_d = nc.dram_tensor("gt", [B, s_core], I32, kind="ExternalInput")
    out_d = nc.dram_tensor("out", [1, 1], F32, kind="ExternalOutput")

    with tile.TileContext(nc) as tc:
        with (
            tc.tile_pool(name="io", bufs=2) as io,
            tc.tile_pool(name="psum", bufs=1, space="PSUM") as pp,
            tc.tile_pool(name="small", bufs=1) as small,
            tc.tile_pool(name="dram", bufs=1, space="DRAM") as dram,
        ):
            acc = {}
            for b in range(B):
                for j in range(n_cg):
                    acc[(b, j)] = pp.tile([P, 20], F32, tag=f"acc{b}{j}",
                                          name=f"acc{b}{j}")

            # constant selector: sel[p, k] = 1 if p % 32 == k (folds the
            # n_cg column-group copies of class k when contracted on PE)
            sel_np = np.zeros((P, K), np.float32)
            for g in range(4):
                for k in range(K):
                    sel_np[32 * g + k, k] = 1.0
            sel_d = nc.inline_tensor(sel_np, name="sel_const")
            sel = small.tile([P, K], F32, name="sel")
            nc.sync.dma_start(sel[:, :], sel_d.ap())

            for b in range(B):
                for blk in range(nblk):
                    f0 = blk * fb
                    dt = io.tile([P, NCOL * fb], F32, tag="dt",
                                 name=f"dt_{b}_{blk}")
                    dtv = dt.rearrange("p (n t) -> p n t", n=NCOL)
                    nc.sync.dma_start(
                        dtv[:, 0:C, :],
                        src.ap()[b].rearrange("c (p t) -> p c t", p=P)[:, :, f0:f0 + fb])
                    nc.sync.dma_start(
                        dtv[:, C:CD, :],
                        tgt.ap()[b].rearrange("c (p t) -> p c t", p=P)[:, :, f0:f0 + fb])
                    nc.vector.memset(dt[:, CD * fb:NCOL * fb], 1.0)

                    gti = io.tile([P, fb], I32, tag="gti", name=f"gti_{b}_{blk}")
                    nc.sync.dma_start(
                        gti[:, :],
                        gti_d.ap()[b].rearrange("(p t) -> p t", p=P)[:, f0:f0 + fb])
                    gtf = io.tile([P, fb], F32, tag="gtf", name=f"gtf_{b}_{blk}")
                    nc.vector.tensor_copy(gtf[:, :], gti[:, :])

                    oh = io.tile([P, K * fb], F32, tag="oh", name=f"oh_{b}_{blk}")
                    for k in range(K):
                        nc.vector.tensor_scalar(
                            oh[:, k * fb:(k + 1) * fb], gtf[:, :],
                            float(k), None, op0=OP.is_equal)
                    ohv = oh.rearrange("p (k t) -> p k t", k=K)

                    for f in range(fb):
                        j = f % n_cg
                        a = acc[(b, j)]
                        first = blk == 0 and f < n_cg
                        last = blk == nblk - 1 and f >= fb - n_cg
                        nc.tensor.matmul(
                            a[32 * j:32 * j + K, 0:NCOL],
                            ohv[:, :, f], dtv[:, :, f],
                            start=first, stop=last,
                            tile_position=(0, 32 * j))

            # ---- partials [128, 20]: rows 32j..32j+4 = col-group j;
            #      cols b*10..b*10+8 = (4 src, 4 tgt, count) for batch b ----
            part = small.tile([P, 20], F32, name="part")
            nc.vector.memset(part[:, :], 0.0)
            for b in range(B):
                for j in range(n_cg):
                    nc.vector.tensor_copy(
                        part[32 * j:32 * j + K, b * 10:b * 10 + NCOL],
                        acc[(b, j)][32 * j:32 * j + K, 0:NCOL])

            cc_in = dram.tile([P, 20], F32, name="cc_in")
            cc_out = dram.tile([P * N_CORES, 20], F32, addr_space="Shared",
                               name="cc_out")
            nc.sync.dma_start(cc_in[:, :], part[:, :])
            nc.gpsimd.collective_compute(
                "AllGather", OP.bypass,
                replica_groups=[list(range(N_CORES))],
                ins=[cc_in.opt()], outs=[cc_out.opt()])

            # raw[p, r*20+col] = cc_out[r*128+p, col]
            raw = small.tile([P, N_CORES * 20], F32, name="raw")
            nc.sync.dma_start(
                raw[:, :],
                cc_out.rearrange("(r p) col -> p r col", r=N_CORES))
            # reduce over ranks (innermost after view)
            sg_t = small.tile([P, 20], F32, name="sg_t")
            nc.vector.reduce_sum(
                sg_t[:, :],
                raw.rearrange("p (r col) -> p col r", r=N_CORES),
                axis=AX.X)
            # fold the 4 column groups on PE: S5 = sel.T @ sg_t -> [K, 20]
            s5 = acc[(1, 0)]
            nc.tensor.matmul(s5[0:K, 0:20], sel[:, :], sg_t[:, :],
                             start=True, stop=True)
            S = small.tile([K, 20], F32, name="S")
            nc.vector.tensor_copy(S[:, :], s5[0:K, 0:20])

            # ---- tiny epilogue (identical on every core) ----
            nvox = small.tile([K, 1], F32, name="nvox")
            nc.vector.tensor_tensor(nvox[:, :], S[:, CD:CD + 1],
                                    S[:, 10 + CD:10 + CD + 1], op=OP.add)
            nde = small.tile([K, 1], F32, name="nde")
            nc.vector.tensor_scalar_add(nde[:, :], nvox[:, :], EPS)
            rec = small.tile([K, 1], F32, name="rec")
            nc.vector.reciprocal(rec[:, :], nde[:, :])

            # avg[k, (b, st, c)] = S_data * rec / TEMP
            avg = small.tile([K, 2 * CD], F32, name="avg")
            nc.vector.tensor_scalar(
                avg.rearrange("k (b c) -> k b c", b=B),
                S.rearrange("k (b c) -> k b c", b=B)[:, :, 0:CD],
                rec[:, :], 1.0 / TEMP, op0=OP.mult, op1=OP.mult)

            av = avg.rearrange("k (b s c) -> k b s c", b=B, s=2)
            dl = small.tile([K, B * C], F32, name="dl")
            dlv = dl.rearrange("k (b c) -> k b c", b=B)
            nc.vector.tensor_tensor(dlv, av[:, :, 1, :], av[:, :, 0, :],
                                    op=OP.subtract)

            ex = small.tile([K, 2 * CD], F32, name="ex")
            nc.scalar.activation(ex[:, :], avg[:, :],
                                 mybir.ActivationFunctionType.Exp)
            zz = small.tile([K, 4], F32, name="zz")
            nc.vector.reduce_sum(zz[:, :],
                                 ex.rearrange("k (g c) -> k g c", g=4), axis=AX.X)

            prd = small.tile([K, 2 * CD], F32, name="prd")
            prdv = prd.rearrange("k (b s c) -> k b s c", b=B, s=2)
            exv = ex.rearrange("k (b s c) -> k b s c", b=B, s=2)
            nc.vector.tensor_tensor(prdv[:, :, 0, :], exv[:, :, 0, :], dlv,
                                    op=OP.mult)
            nc.vector.tensor_tensor(prdv[:, :, 1, :], exv[:, :, 1, :], dlv,
                                    op=OP.mult)
            aa = small.tile([K, 4], F32, name="aa")
            nc.vector.reduce_sum(aa[:, :],
                                 prd.rearrange("k (g c) -> k g c", g=4), axis=AX.X)

            rz = small.tile([K, 4], F32, name="rz")
            nc.vector.reciprocal(rz[:, :], zz[:, :])
            rr = small.tile([K, 4], F32, name="rr")
            nc.vector.tensor_tensor(rr[:, :], aa[:, :], rz[:, :], op=OP.mult)

            # signs: -w for source cols (g even), +w for target cols (g odd)
            w = 1.0 / (2.0 * B * K)
            sg = small.tile([K, 4], F32, name="sg")
            nc.vector.memset(sg[:, :], -w)
            nc.vector.memset(sg.rearrange("k (b s) -> k b s", b=B)[:, :, 1:2], w)
            rs = small.tile([K, 4], F32, name="rs")
            nc.vector.tensor_tensor(rs[:, :], rr[:, :], sg[:, :], op=OP.mult)
            res = small.tile([K, 1], F32, name="res")
            nc.vector.reduce_sum(res[:, :], rs[:, :], axis=AX.X)

            ones5 = small.tile([K, 1], F32, name="ones5")
            nc.vector.memset(ones5[:, :], 1.0)
            # cross-partition sum of res via PE; reuse a corner of acc(0,0)
            psc = acc[(0, 0)]
            nc.tensor.matmul(psc[0:1, 0:1], res[:, :], ones5[:, :],
                             start=True, stop=True)
            o = small.tile([1, 1], F32, name="o")
            nc.vector.tensor_copy(o[:, :], psc[0:1, 0:1])
            nc.sync.dma_start(out_d.ap(), o[:, :])

    nc.compile()
    return nc


_NC_CACHE = {}


def _get_nc(**kwargs):
    key = tuple(sorted(kwargs.items()))
    if key not in _NC_CACHE:
        _NC_CACHE[key] = _build(**kwargs)
    return _NC_CACHE[key]


def _shard_inputs(source_logits, target_logits, gt):
    src = np.asarray(source_logits, dtype=np.float32)[:, 1:].reshape(B, C, D, H * W)
    tgt = np.asarray(target_logits, dtype=np.float32)[:, 1:].reshape(B, C, D, H * W)
    g = np.asarray(gt).astype(np.int32).reshape(B, D, H * W)
    in_maps = []
    for i in range(N_CORES):
        lo, hi = i * D_CORE, (i + 1) * D_CORE
        in_maps.append({
            "src": np.ascontiguousarray(src[:, :, lo:hi]).reshape(B, C, S_CORE),
            "tgt": np.ascontiguousarray(tgt[:, :, lo:hi]).reshape(B, C, S_CORE),
            "gt": np.ascontiguousarray(g[:, lo:hi]).reshape(B, S_CORE),
        })
    return in_maps


def kernel(source_logits, target_logits, gt):
    nc = _get_nc()
    in_maps = _shard_inputs(source_logits, target_logits, gt)
    res = run_bass_kernel_spmd(nc, in_maps, core_ids=list(range(N_CORES)))
    return np.asarray(res.results[0]["out"], dtype=np.float32)[0, 0].reshape(())


def run_traced(source_logits, target_logits, gt, **trace_kwargs):
    """Used by test.py: same as kernel() but returns (value, BassKernelResults)."""
    nc = _get_nc()
    in_maps = _shard_inputs(source_logits, target_logits, gt)
    res = run_bass_kernel_spmd(nc, in_maps, core_ids=list(range(N_CORES)),
                               trace=True, **trace_kwargs)
    val = np.asarray(res.results[0]["out"], dtype=np.float32)[0, 0].reshape(())
    return val, res
